# revision 40
# baseline (speedup 1.0000x reference)
"""Bahdanau additive attention on Trainium2, data-parallel over batch across 8 NeuronCores.

Per core (one batch element):
  q_projT[u, q] = sum_d Wq[u, d] * query[q, d]          (PE, fp32)
  k_projT[u, k] = sum_d Wk[u, d] * keys[k, d]           (PE, bf16)
  S[u, (q,k)]   = k_projT[u, k] + q_projT[u, q]         (DVE tensor_scalar, 4x bf16)
  T             = tanh(S)                               (ACT, in-place, few giant ops)
  scores[q, k]  = sum_u v[u] * T_q[u, k]                (PE, delta-structured v weights so all
                                                         32 q-rows accumulate into one [32,2048] PSUM tile)
  E = exp(scores); sum via ACT accum                    (no max-sub: |scores| <= sum|v| ~ 13)
  attn          = E / sum                               (DVE recip + scale)
  context[q, d] = (sum_k E[q, k] * keys[k, d]) / sum    (PE transposes + bf16 matmul + DVE scale)

All inputs are pre-laid-out on host so every DMA is a contiguous copy.
"""

import numpy as np
import ml_dtypes

import concourse.bacc as bacc
import concourse.bass as bass
import concourse.mybir as mybir
import concourse.tile as tile
from concourse import bass_utils
from concourse.bass import ts

B, Tq, Tk, D, U = 8, 32, 2048, 256, 256
P = 128
NCORES = 8
NB = Tk // 512  # PSUM banks per score row
KC = Tk // P    # key chunks of 128
F32 = mybir.dt.float32
BF16 = mybir.dt.bfloat16
BF16_NP = ml_dtypes.bfloat16

# query 0 is computed straight from the kproj PSUM tiles via the ACT bias trick
# (no kpT copy / DVE add on its critical path); queries 1..31 go through the
# DVE-pre-add + giant-ACT-op path. Ramp down so the final v-dot drain is short.
Q_GROUPS = [1, 2, 4, 4, 4, 4, 4, 4, 2, 1, 1]
assert sum(Q_GROUPS) == Tq - 1

_NC = None


def _emit(nc: bass.Bass, tc: tile.TileContext):
    af = mybir.ActivationFunctionType

    WP = 2 * U + Tq  # packed weight row: wqT | wkT | qT
    keysT = nc.dram_tensor("keysT", [P, 2, Tk], BF16, kind="ExternalInput")  # [p, dc, k]
    keysN = nc.dram_tensor("keysN", [P, KC, D], BF16, kind="ExternalInput")  # [p, kc, d]
    wpack = nc.dram_tensor("wpack", [P, 2, WP], BF16, kind="ExternalInput")  # [p, dc, wq|wk|q]
    vd = nc.dram_tensor("vd", [P, 2, Tq, Tq], BF16, kind="ExternalInput")   # [p, uc, q, m]
    attn_out = nc.dram_tensor("attn_out", [Tq, Tk], F32, kind="ExternalOutput")
    ctx_out = nc.dram_tensor("ctx_out", [Tq, D], F32, kind="ExternalOutput")

    with (
        tc.tile_pool(name="const", bufs=1) as const,
        tc.tile_pool(name="tpool", bufs=3) as tpool,
        tc.tile_pool(name="spool", bufs=1) as spool,
    ):
        # ---- input loads (all contiguous). Two HWDGE queues exist (SP + ACT);
        # keysT chunks go first in each queue (big packets = fast), the packed
        # weights ride along, keysN (needed only at the end) last. ----
        wpack_sb = const.tile([P, 2, WP], BF16)
        vd_sb = const.tile([P, 2, Tq, Tq], BF16)
        keysT_sb = const.tile([P, 2, Tk], BF16)
        keysN_sb = const.tile([P, KC, D], BF16)
        wqT_sb = wpack_sb[:, :, 0:U]
        wkT_sb = wpack_sb[:, :, U : 2 * U]
        qT_sb = wpack_sb[:, :, 2 * U :]
        nc.sync.dma_start(out=keysT_sb[:, 0, ts(0, Tk // 2)], in_=keysT.ap()[:, 0, ts(0, Tk // 2)])
        nc.sync.dma_start(out=keysT_sb[:, 0, ts(1, Tk // 2)], in_=keysT.ap()[:, 0, ts(1, Tk // 2)])
        nc.scalar.dma_start(out=wpack_sb[:], in_=wpack.ap())
        nc.scalar.dma_start(out=keysT_sb[:, 1, ts(0, Tk // 2)], in_=keysT.ap()[:, 1, ts(0, Tk // 2)])
        nc.scalar.dma_start(out=keysT_sb[:, 1, ts(1, Tk // 2)], in_=keysT.ap()[:, 1, ts(1, Tk // 2)])
        nc.gpsimd.dma_start(out=vd_sb[:], in_=vd.ap())
        nc.scalar.dma_start(out=keysN_sb[:], in_=keysN.ap())

        qpT_sb = const.tile([P, 2, Tq], F32)
        kpT_sb = const.tile([P, 2, Tk], BF16)

        # ---- main loop state ----
        e_bf = spool.tile([Tq, Tk], BF16)
        eT_sb = spool.tile([P, KC, Tq], BF16)
        sum_h = spool.tile([Tq, 2], F32)
        sumexp = spool.tile([Tq, 1], F32)

        with (
            tc.tile_pool(name="ps_head", bufs=4, space="PSUM") as ps_head,
            tc.tile_pool(name="ps_scores", bufs=1, space="PSUM") as ps_scores,
        ):
            scores_ps = ps_scores.tile([Tq, Tk], F32)

            # Emission order is engine-FIFO order, so it is chosen so that the
            # first tanh has the shortest possible dependency chain:
            #   qproj (gated only by the small wpack DMA, PE otherwise idle)
            #   kproj-uc0 (per-bank copies chase the accumulating matmuls)
            #   group-0 uc0 adds + tanh (DVE/ACT go straight to work)
            #   kproj-uc1, group-0 uc0 v-dots, group-0 uc1, remaining groups.
            qp_ps = ps_head.tile([P, 512], F32, tag="mm")
            for uc in range(2):
                for dc in range(2):
                    nc.tensor.matmul(
                        qp_ps[:, ts(uc, Tq)],
                        lhsT=wqT_sb[:, dc, ts(uc, P)],
                        rhs=qT_sb[:, dc, :],
                        start=dc == 0,
                        stop=dc == 1,
                    )
            nc.vector.tensor_copy(qpT_sb[:].rearrange("p a q -> p (a q)"), qp_ps[:, : 2 * Tq])

            def kproj_mms(uc):
                kps = [ps_head.tile([P, 512], F32, tag="mm", name=f"kp_{uc}_{nb}") for nb in range(NB)]
                for dc in range(2):
                    for nb in range(NB):
                        nc.tensor.matmul(
                            kps[nb][:],
                            lhsT=wkT_sb[:, dc, ts(uc, P)],
                            rhs=keysT_sb[:, dc, ts(nb, 512)],
                            start=dc == 0,
                            stop=dc == 1,
                        )
                return kps

            def emit_adds_tanh(q0, g, ucs, s_tile):
                for uc in ucs:
                    for qi in range(g):
                        nc.vector.tensor_scalar_add(
                            s_tile[:, qi, uc, :],
                            kpT_sb[:, uc, :],
                            qpT_sb[:, uc, q0 + qi : q0 + qi + 1],
                        )
                view = s_tile[:, :g, ucs[0], :] if len(ucs) == 1 else s_tile[:, :g, :, :]
                nc.scalar.activation(view, view, af.Tanh)

            def emit_vdots(q0, g, ucs, s_tile):
                for qi in range(g):
                    q = q0 + qi
                    for uc in ucs:
                        first = q == 0 and uc == 0
                        last = q == Tq - 1 and uc == 1
                        for nb in range(NB):
                            nc.tensor.matmul(
                                scores_ps[:, ts(nb, 512)],
                                lhsT=vd_sb[:, uc, q, :],
                                rhs=s_tile[:, qi, uc, ts(nb, 512)],
                                start=first,
                                stop=last,
                            )

            # query 0: per-bank tanh straight from PSUM (ACT bias trick) + v-dots,
            # then kpT casts on DVE for the remaining queries
            s0_tile = tpool.tile([P, 1, 2, Tk], BF16, tag="t")
            for uc in range(2):
                kps = kproj_mms(uc)
                for nb in range(NB):
                    nc.scalar.activation(
                        s0_tile[:, 0, uc, ts(nb, 512)],
                        kps[nb][:],
                        af.Tanh,
                        bias=qpT_sb[:, uc, 0:1],
                    )
                    nc.tensor.matmul(
                        scores_ps[:, ts(nb, 512)],
                        lhsT=vd_sb[:, uc, 0, :],
                        rhs=s0_tile[:, 0, uc, ts(nb, 512)],
                        start=uc == 0,
                        stop=False,
                    )
                for nb in range(NB):
                    nc.vector.tensor_copy(kpT_sb[:, uc, ts(nb, 512)], kps[nb][:])

            q0 = 1
            for gi, g in enumerate(Q_GROUPS):
                s_tile = tpool.tile([P, g, 2, Tk], BF16, tag="t")
                split_uc = gi == len(Q_GROUPS) - 1
                for ucs in [(0,), (1,)] if split_uc else [(0, 1)]:
                    emit_adds_tanh(q0, g, ucs, s_tile)
                    emit_vdots(q0, g, ucs, s_tile)
                q0 += g

            # ---- softmax numerator (no max-subtraction), two k-halves so the
            # xbar transpose + context matmuls pipeline behind the first half
            # and the PE idle gap stays under the HAM re-throttle window ----
            for h in range(2):
                nc.scalar.activation(
                    e_bf[:, ts(h, Tk // 2)],
                    scores_ps[:, ts(h, Tk // 2)],
                    af.Exp,
                    bias=0.0,
                    scale=1.0,
                    accum_out=sum_h[:, h : h + 1],
                )
                eng = nc.sync if h == 0 else nc.scalar
                eng.dma_start_transpose(eT_sb[:, ts(h, KC // 2), :], e_bf[:, ts(h, Tk // 2)])

        nc.vector.tensor_add(sumexp[:], sum_h[:, 0:1], sum_h[:, 1:2])
        rsum = spool.tile([Tq, 1], F32)
        nc.vector.reciprocal(rsum[:], sumexp[:])
        attn_sb = spool.tile([Tq, Tk], F32)
        nc.vector.tensor_scalar_mul(attn_sb[:], e_bf[:], rsum[:])
        nc.sync.dma_start(out=attn_out.ap(), in_=attn_sb[:])

        # ---- context: (E @ keys) * rsum ----
        with tc.tile_pool(name="ps_ctx", bufs=1, space="PSUM") as ps_ctx:
            ctx_ps = ps_ctx.tile([P, 512], F32, tag="mm")
            for kc in range(KC):
                nc.tensor.matmul(
                    ctx_ps[:Tq, :D],
                    lhsT=eT_sb[:, kc, :],
                    rhs=keysN_sb[:, kc, :],
                    start=kc == 0,
                    stop=kc == KC - 1,
                )
            ctx_sb = spool.tile([Tq, D], F32)
            nc.vector.tensor_scalar_mul(ctx_sb[:], ctx_ps[:Tq, :D], rsum[:])
            nc.sync.dma_start(out=ctx_out.ap(), in_=ctx_sb[:])


def build():
    global _NC
    if _NC is None:
        nc = bacc.Bacc("TRN2", debug=False)
        with tile.TileContext(nc) as tc:
            _emit(nc, tc)
        nc.compile()
        _NC = nc
    return _NC


def prep_in_maps(query, keys, Wq, Wk, v):
    query = np.asarray(query, np.float32)
    keys = np.asarray(keys, np.float32)
    Wq = np.asarray(Wq, np.float32)
    Wk = np.asarray(Wk, np.float32)
    v = np.asarray(v, np.float32)

    def chunkT(mat, ncols):
        # [rows(=256), ncols] -> [p, dc, ncols]
        return np.ascontiguousarray(mat.reshape(2, P, ncols).transpose(1, 0, 2))

    wqT = chunkT(Wq.T, U)
    wkT = chunkT(Wk.T, U)
    vd = np.zeros((P, 2, Tq, Tq), np.float32)
    idx = np.arange(Tq)
    for uc in range(2):
        vd[:, uc, idx, idx] = v[uc * P : (uc + 1) * P, None]
    vd = vd.astype(BF16_NP)

    in_maps = []
    for b in range(B):
        wpack = np.concatenate([wqT, wkT, chunkT(query[b].T, Tq)], axis=2).astype(BF16_NP)
        in_maps.append(
            dict(
                keysT=chunkT(keys[b].T, Tk).astype(BF16_NP),
                keysN=np.ascontiguousarray(keys[b].reshape(KC, P, D).transpose(1, 0, 2)).astype(BF16_NP),
                wpack=wpack,
                vd=vd,
            )
        )
    return in_maps


def run(query, keys, Wq, Wk, v, trace=False):
    nc = build()
    in_maps = prep_in_maps(query, keys, Wq, Wk, v)
    res = bass_utils.run_bass_kernel_spmd(nc, in_maps, core_ids=list(range(NCORES)), trace=trace)
    context = np.stack([res.results[c]["ctx_out"] for c in range(NCORES)])
    attn = np.stack([res.results[c]["attn_out"] for c in range(NCORES)])
    return (context, attn), res


def kernel(query, keys, Wq, Wk, v):
    (context, attn), _ = run(query, keys, Wq, Wk, v, trace=False)
    return context, attn


# revision 41
# speedup vs baseline: 1.0089x; 1.0089x over previous
"""Bahdanau additive attention on Trainium2, data-parallel over batch across 8 NeuronCores.

Per core (one batch element):
  q_projT[u, q] = sum_d Wq[u, d] * query[q, d]          (PE, fp32)
  k_projT[u, k] = sum_d Wk[u, d] * keys[k, d]           (PE, bf16)
  S[u, (q,k)]   = k_projT[u, k] + q_projT[u, q]         (DVE tensor_scalar, 4x bf16)
  T             = tanh(S)                               (ACT, in-place, few giant ops)
  scores[q, k]  = sum_u v[u] * T_q[u, k]                (PE, delta-structured v weights so all
                                                         32 q-rows accumulate into one [32,2048] PSUM tile)
  E = exp(scores); sum via ACT accum                    (no max-sub: |scores| <= sum|v| ~ 13)
  attn          = E / sum                               (DVE recip + scale)
  context[q, d] = (sum_k E[q, k] * keys[k, d]) / sum    (PE transposes + bf16 matmul + DVE scale)

All inputs are pre-laid-out on host so every DMA is a contiguous copy.
"""

import numpy as np
import ml_dtypes

import concourse.bacc as bacc
import concourse.bass as bass
import concourse.mybir as mybir
import concourse.tile as tile
from concourse import bass_utils
from concourse.bass import ts

B, Tq, Tk, D, U = 8, 32, 2048, 256, 256
P = 128
NCORES = 8
NB = Tk // 512  # PSUM banks per score row
KC = Tk // P    # key chunks of 128
F32 = mybir.dt.float32
BF16 = mybir.dt.bfloat16
BF16_NP = ml_dtypes.bfloat16

# query 0 is computed straight from the kproj PSUM tiles via the ACT bias trick
# (no kpT copy / DVE add on its critical path); queries 1..31 go through the
# DVE-pre-add + giant-ACT-op path. Ramp down so the final v-dot drain is short.
Q_GROUPS = [1, 2, 4, 4, 4, 4, 4, 4, 2, 1, 1]
assert sum(Q_GROUPS) == Tq - 1

_NC = None


def _emit(nc: bass.Bass, tc: tile.TileContext):
    af = mybir.ActivationFunctionType

    WP = 2 * U + Tq  # packed weight row: wqT | wkT | qT
    keysT = nc.dram_tensor("keysT", [P, 2, Tk], BF16, kind="ExternalInput")  # [p, dc, k]
    keysN = nc.dram_tensor("keysN", [P, KC, D], BF16, kind="ExternalInput")  # [p, kc, d]
    wpack = nc.dram_tensor("wpack", [P, 2, WP], BF16, kind="ExternalInput")  # [p, dc, wq|wk|q]
    vd = nc.dram_tensor("vd", [P, 2, Tq, Tq], BF16, kind="ExternalInput")   # [p, uc, q, m]
    attn_out = nc.dram_tensor("attn_out", [Tq, Tk], F32, kind="ExternalOutput")
    ctx_out = nc.dram_tensor("ctx_out", [Tq, D], F32, kind="ExternalOutput")

    with (
        tc.tile_pool(name="const", bufs=1) as const,
        tc.tile_pool(name="tpool", bufs=3) as tpool,
        tc.tile_pool(name="spool", bufs=1) as spool,
    ):
        # ---- input loads (all contiguous). Two HWDGE queues exist (SP + ACT);
        # keysT chunks go first in each queue (big packets = fast), the packed
        # weights ride along, keysN (needed only at the end) last. ----
        wpack_sb = const.tile([P, 2, WP], BF16)
        vd_sb = const.tile([P, 2, Tq, Tq], BF16)
        keysT_sb = const.tile([P, 2, Tk], BF16)
        keysN_sb = const.tile([P, KC, D], BF16)
        wqT_sb = wpack_sb[:, :, 0:U]
        wkT_sb = wpack_sb[:, :, U : 2 * U]
        qT_sb = wpack_sb[:, :, 2 * U :]
        nc.sync.dma_start(out=keysT_sb[:, 0, :], in_=keysT.ap()[:, 0, :])
        nc.scalar.dma_start(out=keysT_sb[:, 1, :], in_=keysT.ap()[:, 1, :])
        nc.gpsimd.dma_start(out=wpack_sb[:], in_=wpack.ap())
        nc.gpsimd.dma_start(out=vd_sb[:], in_=vd.ap())
        nc.scalar.dma_start(out=keysN_sb[:], in_=keysN.ap())

        qpT_sb = const.tile([P, 2, Tq], F32)
        kpT_sb = const.tile([P, 2, Tk], BF16)

        # ---- main loop state ----
        e_bf = spool.tile([Tq, Tk], BF16)
        eT_sb = spool.tile([P, KC, Tq], BF16)
        sum_h = spool.tile([Tq, 2], F32)
        sumexp = spool.tile([Tq, 1], F32)

        with (
            tc.tile_pool(name="ps_head", bufs=4, space="PSUM") as ps_head,
            tc.tile_pool(name="ps_scores", bufs=1, space="PSUM") as ps_scores,
        ):
            scores_ps = ps_scores.tile([Tq, Tk], F32)

            # Emission order is engine-FIFO order, so it is chosen so that the
            # first tanh has the shortest possible dependency chain:
            #   qproj (gated only by the small wpack DMA, PE otherwise idle)
            #   kproj-uc0 (per-bank copies chase the accumulating matmuls)
            #   group-0 uc0 adds + tanh (DVE/ACT go straight to work)
            #   kproj-uc1, group-0 uc0 v-dots, group-0 uc1, remaining groups.
            qp_ps = ps_head.tile([P, 512], F32, tag="mm")
            for uc in range(2):
                for dc in range(2):
                    nc.tensor.matmul(
                        qp_ps[:, ts(uc, Tq)],
                        lhsT=wqT_sb[:, dc, ts(uc, P)],
                        rhs=qT_sb[:, dc, :],
                        start=dc == 0,
                        stop=dc == 1,
                    )
            nc.vector.tensor_copy(qpT_sb[:].rearrange("p a q -> p (a q)"), qp_ps[:, : 2 * Tq])

            def kproj_mms(uc):
                kps = [ps_head.tile([P, 512], F32, tag="mm", name=f"kp_{uc}_{nb}") for nb in range(NB)]
                for dc in range(2):
                    for nb in range(NB):
                        nc.tensor.matmul(
                            kps[nb][:],
                            lhsT=wkT_sb[:, dc, ts(uc, P)],
                            rhs=keysT_sb[:, dc, ts(nb, 512)],
                            start=dc == 0,
                            stop=dc == 1,
                        )
                return kps

            def emit_adds_tanh(q0, g, ucs, s_tile):
                for uc in ucs:
                    for qi in range(g):
                        nc.vector.tensor_scalar_add(
                            s_tile[:, qi, uc, :],
                            kpT_sb[:, uc, :],
                            qpT_sb[:, uc, q0 + qi : q0 + qi + 1],
                        )
                view = s_tile[:, :g, ucs[0], :] if len(ucs) == 1 else s_tile[:, :g, :, :]
                nc.scalar.activation(view, view, af.Tanh)

            def emit_vdots(q0, g, ucs, s_tile):
                for qi in range(g):
                    q = q0 + qi
                    for uc in ucs:
                        first = q == 0 and uc == 0
                        last = q == Tq - 1 and uc == 1
                        for nb in range(NB):
                            nc.tensor.matmul(
                                scores_ps[:, ts(nb, 512)],
                                lhsT=vd_sb[:, uc, q, :],
                                rhs=s_tile[:, qi, uc, ts(nb, 512)],
                                start=first,
                                stop=last,
                            )

            # query 0: per-bank tanh straight from PSUM (ACT bias trick) + v-dots,
            # then kpT casts on DVE for the remaining queries
            s0_tile = tpool.tile([P, 1, 2, Tk], BF16, tag="t")
            for uc in range(2):
                kps = kproj_mms(uc)
                for nb in range(NB):
                    nc.scalar.activation(
                        s0_tile[:, 0, uc, ts(nb, 512)],
                        kps[nb][:],
                        af.Tanh,
                        bias=qpT_sb[:, uc, 0:1],
                    )
                    nc.tensor.matmul(
                        scores_ps[:, ts(nb, 512)],
                        lhsT=vd_sb[:, uc, 0, :],
                        rhs=s0_tile[:, 0, uc, ts(nb, 512)],
                        start=uc == 0,
                        stop=False,
                    )
                for nb in range(NB):
                    nc.vector.tensor_copy(kpT_sb[:, uc, ts(nb, 512)], kps[nb][:])

            q0 = 1
            for gi, g in enumerate(Q_GROUPS):
                s_tile = tpool.tile([P, g, 2, Tk], BF16, tag="t")
                split_uc = gi == len(Q_GROUPS) - 1
                for ucs in [(0,), (1,)] if split_uc else [(0, 1)]:
                    emit_adds_tanh(q0, g, ucs, s_tile)
                    emit_vdots(q0, g, ucs, s_tile)
                q0 += g

            # ---- softmax numerator (no max-subtraction), two k-halves so the
            # xbar transpose + context matmuls pipeline behind the first half
            # and the PE idle gap stays under the HAM re-throttle window ----
            for h in range(2):
                nc.scalar.activation(
                    e_bf[:, ts(h, Tk // 2)],
                    scores_ps[:, ts(h, Tk // 2)],
                    af.Exp,
                    bias=0.0,
                    scale=1.0,
                    accum_out=sum_h[:, h : h + 1],
                )
                eng = nc.sync if h == 0 else nc.scalar
                eng.dma_start_transpose(eT_sb[:, ts(h, KC // 2), :], e_bf[:, ts(h, Tk // 2)])

        nc.vector.tensor_add(sumexp[:], sum_h[:, 0:1], sum_h[:, 1:2])
        rsum = spool.tile([Tq, 1], F32)
        nc.vector.reciprocal(rsum[:], sumexp[:])
        attn_sb = spool.tile([Tq, Tk], F32)
        nc.vector.tensor_scalar_mul(attn_sb[:], e_bf[:], rsum[:])
        nc.sync.dma_start(out=attn_out.ap(), in_=attn_sb[:])

        # ---- context: (E @ keys) * rsum ----
        with tc.tile_pool(name="ps_ctx", bufs=1, space="PSUM") as ps_ctx:
            ctx_ps = ps_ctx.tile([P, 512], F32, tag="mm")
            for kc in range(KC):
                nc.tensor.matmul(
                    ctx_ps[:Tq, :D],
                    lhsT=eT_sb[:, kc, :],
                    rhs=keysN_sb[:, kc, :],
                    start=kc == 0,
                    stop=kc == KC - 1,
                )
            ctx_sb = spool.tile([Tq, D], F32)
            nc.vector.tensor_scalar_mul(ctx_sb[:], ctx_ps[:Tq, :D], rsum[:])
            nc.sync.dma_start(out=ctx_out.ap(), in_=ctx_sb[:])


def build():
    global _NC
    if _NC is None:
        nc = bacc.Bacc("TRN2", debug=False)
        with tile.TileContext(nc) as tc:
            _emit(nc, tc)
        nc.compile()
        _NC = nc
    return _NC


def prep_in_maps(query, keys, Wq, Wk, v):
    query = np.asarray(query, np.float32)
    keys = np.asarray(keys, np.float32)
    Wq = np.asarray(Wq, np.float32)
    Wk = np.asarray(Wk, np.float32)
    v = np.asarray(v, np.float32)

    def chunkT(mat, ncols):
        # [rows(=256), ncols] -> [p, dc, ncols]
        return np.ascontiguousarray(mat.reshape(2, P, ncols).transpose(1, 0, 2))

    wqT = chunkT(Wq.T, U)
    wkT = chunkT(Wk.T, U)
    vd = np.zeros((P, 2, Tq, Tq), np.float32)
    idx = np.arange(Tq)
    for uc in range(2):
        vd[:, uc, idx, idx] = v[uc * P : (uc + 1) * P, None]
    vd = vd.astype(BF16_NP)

    in_maps = []
    for b in range(B):
        wpack = np.concatenate([wqT, wkT, chunkT(query[b].T, Tq)], axis=2).astype(BF16_NP)
        in_maps.append(
            dict(
                keysT=chunkT(keys[b].T, Tk).astype(BF16_NP),
                keysN=np.ascontiguousarray(keys[b].reshape(KC, P, D).transpose(1, 0, 2)).astype(BF16_NP),
                wpack=wpack,
                vd=vd,
            )
        )
    return in_maps


def run(query, keys, Wq, Wk, v, trace=False):
    nc = build()
    in_maps = prep_in_maps(query, keys, Wq, Wk, v)
    res = bass_utils.run_bass_kernel_spmd(nc, in_maps, core_ids=list(range(NCORES)), trace=trace)
    context = np.stack([res.results[c]["ctx_out"] for c in range(NCORES)])
    attn = np.stack([res.results[c]["attn_out"] for c in range(NCORES)])
    return (context, attn), res


def kernel(query, keys, Wq, Wk, v):
    (context, attn), _ = run(query, keys, Wq, Wk, v, trace=False)
    return context, attn
